# revision 52
# baseline (speedup 1.0000x reference)
"""GCN layer (nn_GCNReg) on 8 Trainium2 NeuronCores.

Strategy (graph/data parallel, per sharding hint):
  - Nodes are partitioned across 8 cores by destination range (49 tiles of
    128 nodes per core). Edges are routed to the core owning their dst and
    sorted by dst; self-loops are excluded from the gather (the device adds
    dinv_loc*x_loc^T via one identity matmul per dst tile instead).
  - Math: out = relu(dinv_dst * ((sum_e dinv_src*x[src_e]) @ W1^T) + b1)
    @ W2^T + b2.  The W1 transform commutes with aggregation, so each core
    only transforms its own 6272 aggregated rows.
  - Per-edge rows of raw f16 x are fetched with dma_gather (<=1024 rows per
    call, desc-gen spread over all 4 SWDGE queues / Q7 core pairs; queue ==
    (Tile-scheduled DMASW lane) % 4 so each lane's semaphores see a single
    queue).  Vector scales the gathered messages by the per-slot dinv[src]
    column (grouped broadcast multiply) and builds one-hot scatter blocks
    with batched is_equal; TensorE accumulates agg^T = M^T S in PSUM.
"""

import sys

import numpy as np

for _p in ("/opt/trn_rl_repo", "/opt/pypackages"):
    if _p not in sys.path:
        sys.path.append(_p)

import concourse.bass as bass
import concourse.tile as tile
from concourse import bacc, mybir
from concourse.tile_rust import add_dep_helper
from concourse.bass_utils import run_bass_kernel_spmd

N = 50000
D = 128
HID = 128
ODIM = 8
CORES = 8
TILE = 128
TPC = 49                      # tiles per core (core 7: 48 real + 1 dummy)
NPC = TPC * TILE              # 6272 nodes per core
NP = CORES * NPC              # 50176 padded node count
LO = 32768                    # int16-safe gather-table split
BATCH_BWS = [512] * 12 + [128]   # 12*512 + 128 = 6272
NBATCH = len(BATCH_BWS)
PAD_DL = 999.0                # dstlocal sentinel: matches no iota column
SGROUP = 16                   # S matrices built per tensor_tensor op

F16 = np.float16
F32 = np.float32


def _preprocess(edge_index):
    """Route/sort edges; build uniform compile-time meta + per-core arrays."""
    src = np.asarray(edge_index[0], dtype=np.int64)
    dst = np.asarray(edge_index[1], dtype=np.int64)
    # self-loops count toward degree but are NOT gathered: the device adds
    # xs[dst] via an identity matmul per dst tile instead.
    order = np.argsort(dst, kind="stable")
    ssrc = src[order].astype(np.int32)
    sdst = dst[order].astype(np.int32)

    counts_e = np.bincount(dst, minlength=NP)
    counts = counts_e + 1                      # + self-loop for degree
    counts[N:] = 1                             # pad nodes -> deg 1
    ptr_deg = np.zeros(NP + 1, dtype=np.int64)
    ptr_deg[1:] = np.cumsum(counts)
    ptrA = ptr_deg[:NP].astype(F32)
    ptrB = ptr_deg[1 : NP + 1].astype(F32)

    # edge-only CSR for routing gathers
    ptr = np.zeros(NP + 1, dtype=np.int64)
    ptr[1:] = np.cumsum(counts_e)

    # per (core, batch, side): src-index list + batch-relative dst list
    per = [[None] * (2 * NBATCH) for _ in range(CORES)]
    for c in range(CORES):
        for b in range(NBATCH):
            base = c * NPC + b * 512
            bw = BATCH_BWS[b]
            lo_e = ptr[base]
            hi_e = ptr[min(base + bw, NP)]
            s = ssrc[lo_e:hi_e]
            dl = (sdst[lo_e:hi_e] - base).astype(np.int32)
            m = s < LO
            per[c][2 * b] = (s[m], dl[m])
            per[c][2 * b + 1] = (s[~m] - LO, dl[~m])

    # uniform chunk counts + mm-entry lists
    meta = {"batches": []}
    nchunk_tot = 0      # gather chunks (columns in msg buffers)
    nidx16_tot = 0
    nmm_tot = 0         # matmul entries == dstloc columns
    for b in range(NBATCH):
        bw = BATCH_BWS[b]
        ent = {"bw": bw, "sides": []}
        for side in range(2):
            cmax = max(len(per[c][2 * b + side][0]) for c in range(CORES))
            cmax = max(cmax, 1)
            k = (cmax + TILE - 1) // TILE
            ent["sides"].append(
                {"cmax": cmax, "k": k, "idx_off16": nidx16_tot}
            )
            nchunk_tot += k
            nidx16_tot += (k * TILE) // 16

        # union dst span per chunk across cores -> per-tile mm entries
        mm = []  # (buf_ci, tile_j, dstloc_col)
        klo = ent["sides"][0]["k"]
        for side in range(2):
            sd = ent["sides"][side]
            k = sd["k"]
            lo_span = np.full(k, np.inf)
            hi_span = np.full(k, -np.inf)
            for c in range(CORES):
                _, dl_l = per[c][2 * b + side]
                n = len(dl_l)
                if n == 0:
                    continue
                nk = (n + TILE - 1) // TILE
                starts = np.arange(nk) * TILE
                mn = np.minimum.reduceat(dl_l, starts)
                mx = np.maximum.reduceat(dl_l, starts)
                lo_span[:nk] = np.minimum(lo_span[:nk], mn)
                hi_span[:nk] = np.maximum(hi_span[:nk], mx)
            for ci in range(k):
                if not np.isfinite(lo_span[ci]):
                    continue   # all-pad chunk on every core: no matmul
                j0 = int(lo_span[ci]) // TILE
                j1 = int(hi_span[ci]) // TILE
                buf_ci = ci if side == 0 else klo + ci
                for j in range(j0, j1 + 1):
                    mm.append((buf_ci, j, nmm_tot))
                    nmm_tot += 1
        ent["mm"] = mm
        meta["batches"].append(ent)
    meta["nchunk"] = nchunk_tot
    meta["nidx16"] = nidx16_tot
    meta["nmm"] = nmm_tot
    meta["maxch"] = max(
        e["sides"][0]["k"] + e["sides"][1]["k"] for e in meta["batches"]
    )

    # per-core packed arrays (dinv[src] is pre-folded into the gathered xs
    # rows host-side, so no per-slot dinv table is needed)
    dstloc = np.full((CORES, 128, nmm_tot), PAD_DL, dtype=F16)
    srcidx = np.full((CORES, 128, nidx16_tot), 0, dtype=np.int16)
    for c in range(CORES):
        for b in range(NBATCH):
            ent = meta["batches"][b]
            dls = []
            for side in range(2):
                sd = ent["sides"][side]
                idx_l, dl_l = per[c][2 * b + side]
                n = len(idx_l)
                k = sd["k"]
                tot = k * TILE
                idx = np.zeros(tot, dtype=np.int16)
                idx[:n] = idx_l.astype(np.int16)
                srcidx[c][:, sd["idx_off16"] : sd["idx_off16"] + tot // 16] = (
                    np.tile(idx.reshape(tot // 16, 16).T, (8, 1))
                )
                dl = np.full(tot, PAD_DL, dtype=F32)
                dl[:n] = dl_l.astype(F32)
                dls.append(dl.reshape(k, TILE))
            dl_all = np.concatenate(dls, axis=0)  # [klo+khi, 128]
            for buf_ci, j, col in ent["mm"]:
                dstloc[c][:, col] = (dl_all[buf_ci] - j * TILE).astype(F16)
    # entries outside [0,128) (other tile's edges / pads) match no iota col
    dstloc[np.logical_or(dstloc < 0, dstloc >= TILE)] = PAD_DL

    return meta, ptrA, ptrB, dstloc, srcidx


def _bc_mid(ap2d, g):
    """[128, W] AP -> [128, g, W] with a step-0 middle dim."""
    return bass.AP(ap2d.tensor, ap2d.offset, [ap2d.ap[0], [0, g], ap2d.ap[1]])


def _build_program(meta, queues=None, do_compile=True):
    """queues: per-gather (issue order) SWDGE queue assignment. Must keep
    each Tile DMASW lane (scheduled-order index % 8) on a single queue —
    see _gather_queue_plan."""
    # 48KB/partition descriptor carveout (~768 descs per SWDGE queue ring):
    # each queue gets one ~272-desc lo-half + one ~146-desc hi-half per
    # 2-batch period, and the next period's gen can start before the prior
    # fully drains -- desc-gen on Pool runs ahead instead of stalling
    # mid-generation on ring space.
    nc = bacc.Bacc("TRN2", target_bir_lowering=False, debug=False,
                   num_devices=CORES, num_swdge_queues=4,
                   dynamic_dma_scratch_size=49152)
    dt = mybir.dt

    xf = nc.dram_tensor("xf", [NP, D], dt.float16, kind="ExternalInput")
    ident_d = nc.dram_tensor("ident", [128, 128], dt.float16, kind="ExternalInput")
    xflT_d = nc.dram_tensor("xflT", [128, TPC * D], dt.float16,
                            kind="ExternalInput")
    dinvb_d = nc.dram_tensor("dinvb", [128, NPC], dt.float16,
                             kind="ExternalInput")
    dstloc_d = nc.dram_tensor("dstloc", [128, meta["nmm"]], dt.float16,
                              kind="ExternalInput")
    srcidx_d = nc.dram_tensor("srcidx", [128, meta["nidx16"]], dt.int16,
                              kind="ExternalInput")
    iota_d = nc.dram_tensor("iota", [128, 128], dt.float16, kind="ExternalInput")
    w1t_d = nc.dram_tensor("w1t", [D, HID], dt.float32, kind="ExternalInput")
    b1_d = nc.dram_tensor("b1c", [HID, 1], dt.float32, kind="ExternalInput")
    w2t_d = nc.dram_tensor("w2t", [HID, ODIM], dt.float16, kind="ExternalInput")
    b2_d = nc.dram_tensor("b2c", [ODIM, 1], dt.float32, kind="ExternalInput")
    out_d = nc.dram_tensor("out", [ODIM, NPC], dt.float32, kind="ExternalOutput")

    with tile.TileContext(nc) as tc:
        with (
            tc.tile_pool(name="const", bufs=1) as cpool,
            tc.tile_pool(name="ptr", bufs=1) as ppool,
            tc.tile_pool(name="msg", bufs=4) as msg_pool,
            tc.tile_pool(name="smat", bufs=4) as s_pool,
            tc.tile_pool(name="eptmp", bufs=2) as ep_pool,
            tc.tile_pool(name="psA", bufs=3, space="PSUM") as psA,
            tc.tile_pool(name="psZ", bufs=2, space="PSUM") as psZ,
            tc.tile_pool(name="psO", bufs=2, space="PSUM") as psO,
        ):
            # ---- warmup: a throwaway 128-idx gather issued before anything
            # else. Its idx tile comes from an SBUF memset (no DMA wait), so
            # the one-time ~6us GPSIMD ext-isa IRAM load overlaps the input
            # DMAs instead of delaying the first real gather.
            warm_idx = cpool.tile([128, 8], dt.int16, tag="warmidx")
            nc.gpsimd.memset(warm_idx[:], 0)
            warm_out = cpool.tile([128, 1, D], dt.float16, tag="warmout")

            # ---- constants in ----
            # srcidx loads per batch on the sync HWDGE ring (so batch-0
            # gathers wait only for their own slice); all other inputs go
            # through the second HWDGE ring (ACT sequencer) to keep the
            # sync ring clear.
            idx_t = cpool.tile([128, meta["nidx16"]], dt.int16, tag="srcidx")
            # slices load in PROCESSING order (small batch 12 first, matching
            # the batch loop below) so the first gathers' indices arrive first
            for bi in [NBATCH - 1] + list(range(NBATCH - 1)):
                i0 = meta["batches"][bi]["sides"][0]["idx_off16"]
                s1 = meta["batches"][bi]["sides"][1]
                i1 = s1["idx_off16"] + (s1["k"] * TILE) // 16
                nc.sync.dma_start(idx_t[:, i0:i1], srcidx_d.ap()[:, i0:i1])
            iota_t = cpool.tile([128, 128], dt.float16, tag="iota")
            nc.scalar.dma_start(iota_t[:], iota_d.ap())
            ident_t = cpool.tile([128, 128], dt.float16, tag="ident")
            nc.scalar.dma_start(ident_t[:], ident_d.ap())
            w1t_t = cpool.tile([D, HID], dt.float32, tag="w1t")
            nc.scalar.dma_start(w1t_t[:], w1t_d.ap())
            b1_t = cpool.tile([HID, 1], dt.float32, tag="b1")
            nc.scalar.dma_start(b1_t[:], b1_d.ap())
            w2t_t = cpool.tile([HID, ODIM], dt.float16, tag="w2t")
            nc.scalar.dma_start(w2t_t[:], w2t_d.ap())
            b2_t = cpool.tile([ODIM, 1], dt.float32, tag="b2")
            nc.scalar.dma_start(b2_t[:], b2_d.ap())
            dstloc_t = cpool.tile([128, meta["nmm"]], dt.float16, tag="dstloc")
            nc.scalar.dma_start(dstloc_t[:], dstloc_d.ap())
            zeros_t = cpool.tile([1, 512], dt.float16, tag="zeros")
            nc.vector.memset(zeros_t[:], 0.0)

            # dst-side dinv + self-loop xloc tiles are declared here but
            # loaded after batch 0's gathers are issued (see _late_loads):
            # they are first consumed at ~45us / ~30us, and issuing their
            # DMA up front delays the first gather's srcidx feed. xflT holds
            # the pre-scaled local rows (dinv*x), used directly by the
            # self-loop identity matmuls.
            dinvB_t = cpool.tile([128, NPC], dt.float16, tag="dinvB")
            xloc_t = ppool.tile([128, TPC, D], dt.float16, tag="xloc")

            def _late_loads():
                nc.scalar.dma_start(dinvB_t[:], dinvb_d.ap())
                nc.scalar.dma_start(
                    xloc_t[:], xflT_d.ap().rearrange("p (a d) -> p a d", d=D)
                )

            # ---- phase 2: gathers + one-hot scatter matmuls + epilogue ----
            # gathers read raw xf; dinv[src] is folded into the S values.
            out_acc = cpool.tile([ODIM, NPC], dt.float32, tag="outacc")
            xs_lo_ap = xf.ap()[0:LO, :]
            xs_hi_ap = xf.ap()[LO:NP, :]
            gq = 0  # gather issue counter
            gather_names = []

            wg = nc.gpsimd.dma_gather(
                out_ap=warm_out[:],
                in_ap=xs_lo_ap,
                idxs_ap=warm_idx[:],
                num_idxs=128,
                num_idxs_reg=128,
                elem_size=D,
                single_packet=True,
                queue_num=queues[gq] if queues else 0,
            )
            gather_names.append(wg.ins.name)
            gq += 1

            # small 128-dst batch FIRST so the kernel ends on a regular
            # 512-batch: the tail is the last batch's serial consumer chain,
            # and the small batch's chain otherwise dangles off the very end
            for bi, b in enumerate([NBATCH - 1] + list(range(NBATCH - 1))):
                ent = meta["batches"][b]
                bw = ent["bw"]
                klo = ent["sides"][0]["k"]
                buf = msg_pool.tile([128, meta["maxch"], D], dt.float16,
                                    tag="msg")
                # Each side is split into ~equal multi-packet gathers of at
                # most 7 chunks (<=56 descs/engine, under the 64-desc ring
                # window, so desc-gen never throttles to drain rate). A full
                # batch yields 8 calls (5 lo + 3 hi); sizes are paired
                # big+small so each lane pair -- one SWDGE queue under the
                # (sched_pos % 8)//2 map -- carries ~equal bytes (the
                # per-queue serial drain is the critical path).
                parts = []
                for side, c0 in ((0, 0), (1, klo)):
                    k = ent["sides"][side]["k"]
                    n = -(-k // 7)
                    base, rem = divmod(k, n)
                    p0 = 0
                    for j in range(n):
                        pk = base + (1 if j < rem else 0)
                        parts.append((pk, side, c0, p0))
                        p0 += pk
                parts.sort(key=lambda t: -t[0])
                order = []
                for j in range((len(parts) + 1) // 2):
                    order.append(parts[j][1:] + (parts[j][0],))
                    jj = len(parts) - 1 - j
                    if jj > j:
                        order.append(parts[jj][1:] + (parts[jj][0],))
                for side, c0, p0, pk in order:
                    sd = ent["sides"][side]
                    off = sd["idx_off16"] + (p0 * TILE) // 16
                    g = nc.gpsimd.dma_gather(
                        out_ap=buf[:, c0 + p0 : c0 + p0 + pk, :],
                        in_ap=xs_lo_ap if side == 0 else xs_hi_ap,
                        idxs_ap=idx_t[:, off : off + (pk * TILE) // 16],
                        num_idxs=pk * TILE,
                        num_idxs_reg=pk * TILE,
                        elem_size=D,
                        # single_packet caps a call at 1024 idxs (64 descs =
                        # one packet per engine); multi-packet lifts that.
                        single_packet=(pk * TILE <= 1024),
                        queue_num=queues[gq] if queues else 0,
                    )
                    gather_names.append(g.ins.name)
                    gq += 1
                if b == 0:
                    _late_loads()

                # zeros-init opens the accumulation group (full bank), then
                # self-loop identity matmuls accumulate xs_local^T per tile.
                # Explicit deps pin them between the init and the stop mm.
                agg_ps = psA.tile([128, 512], dt.float32, tag="agg")
                init_mm = nc.tensor.matmul(
                    out=agg_ps[:], lhsT=zeros_t[:, :128], rhs=zeros_t[:],
                    start=True, stop=False, skip_group_check=True,
                )
                mm = ent["mm"]
                nmm = len(mm)
                ntile = bw // TILE
                id_mms = []
                for j in range(ntile):
                    im = nc.tensor.matmul(
                        out=agg_ps[:, j * TILE : (j + 1) * TILE],
                        lhsT=xloc_t[:, b * 4 + j, :],
                        rhs=ident_t[:],
                        start=False,
                        stop=(nmm == 0 and j == ntile - 1),
                        skip_group_check=True,
                    )
                    add_dep_helper(im.ins, init_mm.ins,
                                   reason="identity after init")
                    id_mms.append(im)
                for g0 in range(0, nmm, SGROUP):
                    gn = min(SGROUP, nmm - g0)
                    col0 = mm[g0][2]
                    s_t = s_pool.tile([128, SGROUP, TILE], dt.float16,
                                      tag="smat")
                    nc.vector.tensor_tensor(
                        out=s_t[:, :gn, :],
                        in0=_bc_mid(iota_t[:], gn),
                        in1=dstloc_t[:, col0 : col0 + gn].to_broadcast(
                            [128, gn, TILE]
                        ),
                        op=mybir.AluOpType.is_equal,
                    )
                    for gi in range(gn):
                        buf_ci, j, _ = mm[g0 + gi]
                        emm = nc.tensor.matmul(
                            out=agg_ps[:, j * TILE : (j + 1) * TILE],
                            lhsT=buf[:, buf_ci, :],
                            rhs=s_t[:, gi, :],
                            start=False,
                            stop=(g0 + gi == nmm - 1),
                            skip_group_check=True,
                        )
                        if g0 + gi == nmm - 1:
                            for im in id_mms:
                                add_dep_helper(emm.ins, im.ins,
                                               reason="stop after identity")

                # epilogue for this batch
                agg_sb = ep_pool.tile([128, 512], dt.float32, tag="aggsb")
                nc.scalar.copy(agg_sb[:, :bw], agg_ps[:, :bw])
                z_ps = psZ.tile([128, 512], dt.float32, tag="z")
                nc.tensor.matmul(out=z_ps[:, :bw], lhsT=w1t_t[:],
                                 rhs=agg_sb[:, :bw], start=True, stop=True)
                z2_sb = ep_pool.tile([128, 512], dt.float32, tag="z2")
                nc.vector.tensor_tensor(
                    out=z2_sb[:, :bw],
                    in0=z_ps[:, :bw],
                    in1=dinvB_t[:, b * 512 : b * 512 + bw],
                    op=mybir.AluOpType.mult,
                )
                h_sb = ep_pool.tile([128, 512], dt.float16, tag="h")
                nc.scalar.activation(h_sb[:, :bw], z2_sb[:, :bw],
                                     mybir.ActivationFunctionType.Relu,
                                     bias=b1_t[:])
                o_ps = psO.tile([ODIM, 512], dt.float32, tag="o")
                nc.tensor.matmul(out=o_ps[:, :bw], lhsT=w2t_t[:],
                                 rhs=h_sb[:, :bw], start=True, stop=True)
                # bias add on the (mostly idle) ACT engine: Identity(in + b2)
                nc.scalar.activation(
                    out_acc[:, b * 512 : b * 512 + bw],
                    o_ps[:, :bw],
                    mybir.ActivationFunctionType.Identity,
                    bias=b2_t[:],
                )
                # stream this batch's output slice out immediately
                nc.sync.dma_start(
                    out_d.ap()[:, b * 512 : b * 512 + bw],
                    out_acc[:, b * 512 : b * 512 + bw],
                )

    nc._gather_issue_names = gather_names
    if do_compile:
        nc.compile()
    return nc


def _sched_gather_order(nc):
    """Gather instruction names in final (post-Tile-scheduling) stream order
    == the order Tile's DMASW lane round-robin sees them."""
    order = []
    for blk in nc.m.functions[0].blocks:
        for ins in blk.instructions:
            if isinstance(ins, mybir.InstDMAGatherAnt):
                order.append(ins.name)
    return order


def _gather_queue_plan(meta):
    """Two-pass build: learn the scheduled gather order, then assign each
    gather queue = (sched_pos % 8) // 2 so each of Tile's 8 DMASW semaphore
    lanes is only ever incremented by one SWDGE queue (ucode shadow-sem
    rule).  The //2 map pairs one big (lo-side) and one small (hi-side)
    gather per queue per batch, balancing bytes across the 4 queues."""
    probe = _build_program(meta, queues=None, do_compile=False)
    sched = {name: i for i, name in enumerate(_sched_gather_order(probe))}
    issue = probe._gather_issue_names
    assert len(sched) == len(issue)
    queues = [(sched[name] % 8) // 2 for name in issue]
    nc = _build_program(meta, queues=queues, do_compile=True)
    # verify determinism: scheduled order must align lanes with queues
    final = _sched_gather_order(nc)
    name_to_queue = dict(zip(nc._gather_issue_names, queues))
    for pos, name in enumerate(final):
        assert name_to_queue[name] == (pos % 8) // 2, (
            f"gather {name} at sched pos {pos} has queue "
            f"{name_to_queue[name]}, want {(pos % 8) // 2}"
        )
    return nc


_CACHE = {}
last_results = None


def kernel(x, edge_index, W1, b1, W2, b2):
    import os

    meta, ptrA, ptrB, dstloc, srcidx = _preprocess(edge_index)

    dinv_full = np.sqrt(1.0 / (ptrB - ptrA).astype(np.float64))

    # pre-scale rows by dinv[src] host-side: gathered messages and the
    # self-loop tiles then need no on-device dinv[src] multiply.
    xf = np.zeros((NP, D), dtype=F16)
    xf[:N] = (np.asarray(x, dtype=F32)
              * dinv_full[:N, None].astype(F32)).astype(F16)
    iota = np.broadcast_to(np.arange(128, dtype=F16), (128, 128)).copy()
    ident = np.eye(128, dtype=F16)
    w1t = np.asarray(W1, dtype=F32).T.copy()              # [D, HID]
    b1c = np.asarray(b1, dtype=F32).reshape(HID, 1)
    w2t = np.asarray(W2, dtype=F32).T.astype(F16).copy()  # [HID, ODIM]
    b2c = np.asarray(b2, dtype=F32).reshape(ODIM, 1)

    key = tuple(
        (e["bw"], tuple(e["mm"]))
        + tuple((sd["cmax"], sd["k"]) for sd in e["sides"])
        for e in meta["batches"]
    )
    if key not in _CACHE:
        _CACHE[key] = _gather_queue_plan(meta)
    nc = _CACHE[key]

    in_maps = []
    for c in range(CORES):
        sl = slice(c * NPC, (c + 1) * NPC)
        in_maps.append(
            {
                "xf": xf,
                "dinvb": np.broadcast_to(
                    dinv_full[sl].astype(F16), (128, NPC)).copy(),
                "xflT": xf[sl].reshape(TPC, 128, D).transpose(1, 0, 2)
                          .reshape(128, TPC * D).copy(),
                "ident": ident,
                "dstloc": dstloc[c],
                "srcidx": srcidx[c],
                "iota": iota,
                "w1t": w1t,
                "b1c": b1c,
                "w2t": w2t,
                "b2c": b2c,
            }
        )

    trace = bool(os.environ.get("GCN_TRACE"))
    res = run_bass_kernel_spmd(
        nc, in_maps, core_ids=list(range(CORES)), trace=trace
    )
    global last_results
    last_results = res
    big = np.concatenate([res.results[c]["out"] for c in range(CORES)], axis=1)
    return np.ascontiguousarray(big[:, :N].T).astype(F32)

